# revision 42
# baseline (speedup 1.0000x reference)
"""Trainium2 Bass kernel for nn_Attention_53652731461991.

Full multi-head attention block (qkv -> per-head LN(q,k) -> softmax(QK^T) -> @V -> proj)
for x [2, 2048, 1024], 16 heads, hd=64, fp32.

Sharding: tensor-parallel over heads. Each of the 8 cores computes 2 heads
end-to-end (qkv column slice, per-head LN, attention, and its row-slice of the
output projection), producing a partial [4096, 1024] projection output. The
host unshards by summing the 8 row-split partials (standard TP row-parallel
combine) and adding b_proj.

On-core dataflow is fully "transposed" (tokens on the free axis):
  qkvT [384, 4096] = w_c^T @ x^T   (x^T is prepared host-side, a pure layout op)
  LN over the 64-dim head axis via PE ones-matmul stats, K=1 ones-matmul
    partition-broadcast of the per-token scale/shift, DVE apply
  S^T[k,q] = k_hat^T-tile @ q_hat  (two heads on partition halves 0:64/64:128,
    both written into one 2-bank psum tile)
  P = exp(S^T) (no max-subtraction; logits are O(1) after LN), one ACT op per
    2-bank tile
  O^T[65,q] += [V|1]^T-tile @ P    (ones column gives the softmax denominator)
  y_partial = (O^T/denom)^T @ w_proj[head rows]  (K=128: both heads stacked)

All matmuls run as float32r (FP22 multiplies, fp32 accumulate) with moving
free dim >= 256 for full PE rate.
"""

import os
import sys

for _p in ("/opt/trn_rl_repo",):
    if _p not in sys.path and os.path.isdir(_p):
        sys.path.insert(0, _p)

import numpy as np
from contextlib import ExitStack

import concourse.bass as bass
import concourse.bacc as bacc
import concourse.tile as tile
import concourse.mybir as mybir
from concourse.bass_utils import run_bass_kernel_spmd

F32 = mybir.dt.float32
F32R = mybir.dt.float32r
AF = mybir.ActivationFunctionType
OP = mybir.AluOpType

P = 128
C = 1024          # model dim
KO = C // P       # 8 k-subtiles
B = 2
SEQ = 2048
TOKS = B * SEQ    # 4096
TB = 512          # token block (phase 1 / q blocks)
NTB = TOKS // TB  # 8
HD = 64
NQB = SEQ // TB   # 4 q-blocks per batch
NKT = SEQ // P    # 16 k-tiles per batch
NQT = SEQ // P    # 16 q row-tiles per batch (proj)
EPS = 1e-5
NCORES = 8


def _r(ap):
    return ap.bitcast(F32R)


def _emit(tc):
    nc = tc.nc
    xT = nc.dram_tensor("xT", [NTB, P, KO, TB], F32, kind="ExternalInput")
    w = nc.dram_tensor("w", [P, KO, 384], F32, kind="ExternalInput")
    bqkv = nc.dram_tensor("bqkv", [P, 3], F32, kind="ExternalInput")
    wp = nc.dram_tensor("wp", [P, C], F32, kind="ExternalInput")
    # aux: col 0 ones, col 1 eps; two [128, 66] stats selectors:
    # q-sel (cols 2:68): head A rows -> out row 0, B -> row 1, rest zero
    # k-sel (cols 68:134): head A rows -> out row 64, B -> row 65, rest zero
    aux = nc.dram_tensor("aux", [P, 134], F32, kind="ExternalInput")
    # rows2[h, s, :]: s=0 all-ones; s in 1..4: head-padded g/be rows
    # (row = [val|0] for head A, [0|val] for B) at partitions {0,1} and {64,65}
    rows2 = nc.dram_tensor("rows2", [66, 5, P], F32, kind="ExternalInput")
    bq2 = nc.dram_tensor("bq2", [P, 1], F32, kind="ExternalInput")
    bk2 = nc.dram_tensor("bk2", [P, 1], F32, kind="ExternalInput")
    idd = nc.dram_tensor("idd", [P, P], F32, kind="ExternalInput")      # identity
    y = nc.dram_tensor("y", [B, NQT, P, C], F32, kind="ExternalOutput")

    with ExitStack() as ctx:
        const = ctx.enter_context(tc.tile_pool(name="const", bufs=1))
        resid = ctx.enter_context(tc.tile_pool(name="resid", bufs=1))
        xst = ctx.enter_context(tc.tile_pool(name="xst", bufs=2))
        scratch = ctx.enter_context(tc.tile_pool(name="scratch", bufs=4))
        bcast = ctx.enter_context(tc.tile_pool(name="bcast", bufs=3))
        st1 = ctx.enter_context(tc.tile_pool(name="st1", bufs=4))
        stb = ctx.enter_context(tc.tile_pool(name="stb", bufs=3))
        ysb = ctx.enter_context(tc.tile_pool(name="ysb", bufs=3))
        psa = ctx.enter_context(tc.tile_pool(name="psa", bufs=2, space="PSUM"))
        pso = ctx.enter_context(tc.tile_pool(name="pso", bufs=2, space="PSUM"))
        psq = ctx.enter_context(tc.tile_pool(name="psq", bufs=1, space="PSUM"))

        # ---- constants ----
        w_sb = const.tile([P, KO, 384], F32)
        nc.sync.dma_start(_r(w_sb[:]), _r(w[:, :, :]))
        wp_sb = const.tile([P, C], F32)
        nc.sync.dma_start(_r(wp_sb[:]), _r(wp[:, :]))
        b_sb = const.tile([P, 3], F32)
        nc.sync.dma_start(b_sb[:], bqkv[:, :])
        ident = const.tile([P, P], F32)
        nc.sync.dma_start(ident[:], idd[:, :])
        aux_sb = const.tile([P, 134], F32)
        nc.sync.dma_start(_r(aux_sb[:]), _r(aux[:, :]))
        ones = aux_sb[:, 0:1]
        rows_sb = const.tile([66, 5, P], F32)
        nc.sync.dma_start(_r(rows_sb[:]), _r(rows2[:, :, :]))
        bq2_sb = const.tile([P, 1], F32)
        nc.sync.dma_start(bq2_sb[:], bq2[:, :])
        bk2_sb = const.tile([P, 1], F32)
        nc.sync.dma_start(bk2_sb[:], bk2[:, :])

        # ---- residents ----
        qT = resid.tile([P, TOKS], F32)   # heads 2c (rows 0:64) and 2c+1 (64:128)
        kT = resid.tile([P, TOKS], F32)
        vT = resid.tile([P, TOKS], F32)
        vtok = resid.tile([P, B * 2, NKT, HD + 1], F32)  # token-major V + ones col
        nc.vector.tensor_copy(_r(vtok[:, :, :, HD:HD + 1]),
                              ones.to_broadcast((P, B * 2, NKT, 1)))
        OT2 = resid.tile([P, B, SEQ], F32)  # normalized attention out, heads stacked

        # ---- phase 1: qkvT = w^T @ xT, biased; block-local LN stats+apply ----
        def emit_tb(tb):
            ts = slice(tb * TB, (tb + 1) * TB)
            xc = xst.tile([P, KO, TB], F32)
            nc.sync.dma_start(_r(xc[:, 0:KO // 2, :]), _r(xT[tb, :, 0:KO // 2, :]))
            nc.sync.dma_start(_r(xc[:, KO // 2:KO, :]), _r(xT[tb, :, KO // 2:KO, :]))
            for ct, dest in ((0, qT), (1, kT), (2, vT)):
                ps = psq.tile([P, TB], F32, tag="q")
                for ko in range(KO):
                    nc.tensor.matmul(
                        ps[:],
                        lhsT=_r(w_sb[:, ko, ct * P:(ct + 1) * P]),
                        rhs=_r(xc[:, ko, :]),
                        start=(ko == 0),
                        stop=(ko == KO - 1),
                    )
                dslc = dest[:, ts] if ct == 2 else _r(dest[:, ts])
                nc.scalar.activation(dslc, ps[:], AF.Identity,
                                     bias=b_sb[:, ct:ct + 1], scale=1.0)
            # Block-local LN of q and k. All four (tensor, head) instances are
            # partition-packed into one 2-bank stats psum tile via M=32
            # replicated ones matmuls (rows 0:32 qA | 32:64 qB | 64:96 kA |
            # 96:128 kB; free slots mu|msq), so the whole stats pipeline runs
            # as a handful of full-width DVE ops.
            sqq = scratch.tile([P, TB], F32, tag="sc")
            nc.scalar.activation(_r(sqq[:]), qT[:, ts], AF.Square)
            sqk = scratch.tile([P, TB], F32, tag="sc")
            nc.scalar.activation(_r(sqk[:]), kT[:, ts], AF.Square)
            sel_q = aux_sb[:, 2:68]
            sel_k = aux_sb[:, 68:134]
            stqk = psa.tile([66, 2, TB], F32, tag="a2")
            nc.tensor.matmul(stqk[:, 0, :], lhsT=_r(sel_q), rhs=_r(qT[:, ts]),
                             start=True, stop=False)
            nc.tensor.matmul(stqk[:, 0, :], lhsT=_r(sel_k), rhs=_r(kT[:, ts]),
                             start=False, stop=True)
            nc.tensor.matmul(stqk[:, 1, :], lhsT=_r(sel_q), rhs=_r(sqq[:]),
                             start=True, stop=False)
            nc.tensor.matmul(stqk[:, 1, :], lhsT=_r(sel_k), rhs=_r(sqk[:]),
                             start=False, stop=True)
            t_all = stb.tile([66, 2, TB], F32, tag="st")   # mu|msq -> nb|rs
            t_sq = stb.tile([66, TB], F32, tag="st2")
            nc.scalar.activation(_r(t_all[:, :, :]), stqk[:, :, :], AF.Identity,
                                 bias=0.0, scale=1.0 / HD)
            nc.vector.tensor_tensor(_r(t_sq[:]), t_all[:, 0, :], t_all[:, 0, :],
                                    OP.mult)
            nc.vector.tensor_tensor(_r(t_all[:, 1, :]), t_all[:, 1, :], t_sq[:],
                                    OP.subtract)
            nc.scalar.activation(_r(t_all[:, 1, :]), t_all[:, 1, :], AF.Sqrt,
                                 bias=aux_sb[0:66, 1:2])
            with nc.allow_low_precision(reason="fp32r feed to PE broadcast"):
                nc.vector.reciprocal(_r(t_all[:, 1, :]), t_all[:, 1, :])   # rs
            nc.vector.scalar_tensor_tensor(_r(t_all[:, 0, :]), t_all[:, 0, :],
                                           -1.0, t_all[:, 1, :],
                                           OP.mult, OP.mult)               # -mu*rs
            # Per-(partition,token) LN coefficients via K=1 outer-product
            # matmuls, with gamma/beta folded in:
            #   rbnb[:,0,:] = g (x) rs        rbnb[:,1,:] = g (x) nb + be (x) 1
            for src_t, gsl, bev, r0 in ((qT, 1, bq2_sb, 0),
                                        (kT, 3, bk2_sb, 64)):
                rbnb = psa.tile([P, 2, TB], F32, tag="a2",
                                name=f"rbnb_{tb}_{gsl}")
                nc.tensor.matmul(rbnb[:, 0, :],
                                 lhsT=_r(rows_sb[r0:r0 + 2, gsl, :]),
                                 rhs=_r(t_all[r0:r0 + 2, 1, :]),
                                 start=True, stop=True)
                nc.tensor.matmul(rbnb[:, 1, :],
                                 lhsT=_r(rows_sb[r0:r0 + 2, gsl, :]),
                                 rhs=_r(t_all[r0:r0 + 2, 0, :]),
                                 start=True, stop=True)
                tgt = src_t[:, ts]
                nc.vector.tensor_tensor(_r(tgt), tgt, rbnb[:, 0, :], OP.mult)
                nc.vector.scalar_tensor_tensor(_r(tgt), tgt, bev[:, :],
                                               rbnb[:, 1, :], OP.add, OP.add)
            # V transposes for this block's tokens (token-major V for O matmuls)
            vb2 = tb // (NTB // B)
            for h in range(2):
                hb = HD * h
                for kt in range((tb % 4) * 4, (tb % 4) * 4 + 4):
                    kts = slice(vb2 * SEQ + kt * P, vb2 * SEQ + (kt + 1) * P)
                    ps_t = pso.tile([P, HD], F32, tag="o")
                    nc.tensor.transpose(ps_t[:], vT[hb:hb + HD, kts],
                                        ident[hb:hb + HD, hb:hb + HD])
                    nc.vector.tensor_copy(_r(vtok[:, vb2 * 2 + h, kt, 0:HD]),
                                          ps_t[:])

        # ---- phase 2: attention ----
        def emit_attn(b2, qb):
            if True:
                qs = slice(b2 * SEQ + qb * TB, b2 * SEQ + (qb + 1) * TB)
                o_ps = [pso.tile([HD + 1, TB], F32, tag="o", name=f"o_{b2}_{qb}_{hh}")
                        for hh in range(2)]
                for kt in range(NKT):
                    kts = slice(b2 * SEQ + kt * P, b2 * SEQ + (kt + 1) * P)
                    s2 = psa.tile([P, 2, TB], F32, tag="a2")
                    for h in range(2):
                        hb = HD * h
                        nc.tensor.matmul(s2[:, h, :],
                                         lhsT=_r(kT[hb:hb + HD, kts]),
                                         rhs=_r(qT[hb:hb + HD, qs]),
                                         start=True, stop=True)
                    e2 = scratch.tile([P, 2, TB], F32, tag="sc2")
                    nc.scalar.activation(_r(e2[:]), s2[:], AF.Exp)
                    for h in range(2):
                        nc.tensor.matmul(o_ps[h][:],
                                         lhsT=_r(vtok[:, b2 * 2 + h, kt, :]),
                                         rhs=_r(e2[:, h, :]),
                                         start=(kt == 0), stop=(kt == NKT - 1))
                # normalize: reciprocal of denominators, broadcast, scale
                rc0 = st1.tile([1, TB], F32, tag="t1")
                rc1 = st1.tile([1, TB], F32, tag="t1")
                with nc.allow_low_precision(reason="fp32r feed to PE broadcast"):
                    nc.vector.reciprocal(_r(rc0[:]), o_ps[0][HD:HD + 1, :])
                    nc.vector.reciprocal(_r(rc1[:]), o_ps[1][HD:HD + 1, :])
                rbp0 = psa.tile([P, TB], F32, tag="rb", bufs=1)
                nc.tensor.matmul(rbp0[:], lhsT=_r(rows_sb[0:1, 0, :]),
                                 rhs=_r(rc0[:]), start=True, stop=True)
                rbp1 = psa.tile([P, TB], F32, tag="rb", bufs=1)
                nc.tensor.matmul(rbp1[:], lhsT=_r(rows_sb[0:1, 0, :]),
                                 rhs=_r(rc1[:]), start=True, stop=True)
                rb0 = bcast.tile([HD, TB], F32, tag="bc")
                nc.vector.tensor_copy(rb0[:], rbp0[0:HD, :])
                rb1 = bcast.tile([HD, TB], F32, tag="bc")
                nc.vector.tensor_copy(rb1[:], rbp1[0:HD, :])
                nc.vector.tensor_tensor(
                    _r(OT2[0:HD, b2, qb * TB:(qb + 1) * TB]),
                    o_ps[0][0:HD, :], rb0[:], OP.mult)
                ob = ysb.tile([HD, TB], F32, tag="ob")
                nc.vector.tensor_tensor(_r(ob[:]), o_ps[1][0:HD, :],
                                        rb1[:], OP.mult)
                nc.sync.dma_start(
                    _r(OT2[HD:P, b2, qb * TB:(qb + 1) * TB]), _r(ob[:]))

        # ---- phase 3: projection partial ----
        def emit_proj(b2, qt):
            if True:
                yt = ysb.tile([P, C], F32, tag="yt", name=f"yt_{b2}_{qt}")
                for half in range(2):
                    pp = psq.tile([P, TB], F32, tag="q")
                    nc.tensor.matmul(
                        pp[:],
                        lhsT=_r(OT2[:, b2, qt * P:(qt + 1) * P]),
                        rhs=_r(wp_sb[:, half * TB:(half + 1) * TB]),
                        start=True, stop=True)
                    nc.vector.tensor_copy(yt[:, half * TB:(half + 1) * TB], pp[:])
                nc.gpsimd.dma_start(y[b2, qt, :, :], yt[:])

        # ---- interleaved emission: attention(b2=0) woven into phase-1 tail,
        # proj(b2=0) woven into attention(b2=1), so the scheduler can fill
        # engine idle across phase boundaries ----
        for tb in range(NTB):
            emit_tb(tb)
        for b2 in range(B):
            for qb in range(NQB):
                emit_attn(b2, qb)
        for b2 in range(B):
            for qt in range(NQT):
                emit_proj(b2, qt)


_NC_CACHE = None


def build_nc():
    global _NC_CACHE
    if _NC_CACHE is None:
        nc = bacc.Bacc("TRN2", target_bir_lowering=False, debug=False)
        with tile.TileContext(nc) as tc:
            _emit(tc)
        nc.compile()
        _NC_CACHE = nc
    return _NC_CACHE


def make_in_maps(x, w_qkv, b_qkv, g_q, be_q, g_k, be_k, w_proj):
    x2 = np.ascontiguousarray(np.asarray(x, np.float32).reshape(TOKS, C))
    # xT[tb, p, ko, t] = x2[tb*TB + t, ko*128 + p]
    xT_h = np.ascontiguousarray(
        x2.T.reshape(KO, P, NTB, TB).transpose(2, 1, 0, 3))
    w_qkv = np.asarray(w_qkv, np.float32)
    b_qkv = np.asarray(b_qkv, np.float32)
    g_q = np.asarray(g_q, np.float32)
    be_q = np.asarray(be_q, np.float32)
    g_k = np.asarray(g_k, np.float32)
    be_k = np.asarray(be_k, np.float32)
    w_proj = np.asarray(w_proj, np.float32)

    aux_h = np.zeros((P, 134), np.float32)
    aux_h[:, 0] = 1.0
    aux_h[:, 1] = EPS
    aux_h[0:HD, 2] = 1.0         # q-sel: head A -> row 0
    aux_h[HD:P, 3] = 1.0         # q-sel: head B -> row 1
    aux_h[0:HD, 68 + 64] = 1.0   # k-sel: head A -> row 64
    aux_h[HD:P, 68 + 65] = 1.0   # k-sel: head B -> row 65
    rows_h = np.zeros((66, 5, P), np.float32)
    rows_h[:, 0, :] = 1.0
    for s, vec in ((1, g_q / 8.0), (2, be_q / 8.0), (3, g_k), (4, be_k)):
        for r in (0, 64):
            rows_h[r, s, 0:HD] = vec
            rows_h[r + 1, s, HD:P] = vec
    bq2_h = np.ascontiguousarray(np.tile(be_q / 8.0, 2).reshape(P, 1))
    bk2_h = np.ascontiguousarray(np.tile(be_k, 2).reshape(P, 1))
    idd_h = np.ascontiguousarray(np.eye(P, dtype=np.float32))
    in_maps = []
    for c in range(NCORES):
        cs = slice(P * c, P * (c + 1))
        wcat = np.concatenate(
            [w_qkv[:, 0:C][:, cs], w_qkv[:, C:2 * C][:, cs], w_qkv[:, 2 * C:3 * C][:, cs]],
            axis=1)  # [1024, 384]
        w_h = np.ascontiguousarray(wcat.reshape(KO, P, 384).transpose(1, 0, 2))
        bcat = np.concatenate(
            [b_qkv[0:C][cs], b_qkv[C:2 * C][cs], b_qkv[2 * C:3 * C][cs]])
        b_h = np.ascontiguousarray(bcat.reshape(3, P).T)
        wp_h = np.ascontiguousarray(w_proj[cs, :])
        in_maps.append({
            "xT": xT_h, "w": w_h, "bqkv": b_h,
            "wp": wp_h, "aux": aux_h, "rows2": rows_h, "idd": idd_h,
            "bq2": bq2_h, "bk2": bk2_h,
        })
    return in_maps


def kernel(x, w_qkv, b_qkv, g_q, be_q, g_k, be_k, w_proj, b_proj, **run_kwargs):
    in_maps = make_in_maps(x, w_qkv, b_qkv, g_q, be_q, g_k, be_k, w_proj)
    nc = build_nc()
    res = run_bass_kernel_spmd(nc, in_maps, list(range(NCORES)), **run_kwargs)
    acc = np.zeros((TOKS, C), np.float64)
    for r in res.results:
        acc += r["y"].reshape(TOKS, C)
    out = acc + np.asarray(b_proj, np.float32)
    out = out.astype(np.float32).reshape(B, SEQ, C)
    kernel.last_result = res
    return out


# revision 43
# speedup vs baseline: 1.0031x; 1.0031x over previous
"""Trainium2 Bass kernel for nn_Attention_53652731461991.

Full multi-head attention block (qkv -> per-head LN(q,k) -> softmax(QK^T) -> @V -> proj)
for x [2, 2048, 1024], 16 heads, hd=64, fp32.

Sharding: tensor-parallel over heads. Each of the 8 cores computes 2 heads
end-to-end (qkv column slice, per-head LN, attention, and its row-slice of the
output projection), producing a partial [4096, 1024] projection output. The
host unshards by summing the 8 row-split partials (standard TP row-parallel
combine) and adding b_proj.

On-core dataflow is fully "transposed" (tokens on the free axis):
  qkvT [384, 4096] = w_c^T @ x^T   (x^T is prepared host-side, a pure layout op)
  LN over the 64-dim head axis via PE ones-matmul stats, K=1 ones-matmul
    partition-broadcast of the per-token scale/shift, DVE apply
  S^T[k,q] = k_hat^T-tile @ q_hat  (two heads on partition halves 0:64/64:128,
    both written into one 2-bank psum tile)
  P = exp(S^T) (no max-subtraction; logits are O(1) after LN), one ACT op per
    2-bank tile
  O^T[65,q] += [V|1]^T-tile @ P    (ones column gives the softmax denominator)
  y_partial = (O^T/denom)^T @ w_proj[head rows]  (K=128: both heads stacked)

All matmuls run as float32r (FP22 multiplies, fp32 accumulate) with moving
free dim >= 256 for full PE rate.
"""

import os
import sys

for _p in ("/opt/trn_rl_repo",):
    if _p not in sys.path and os.path.isdir(_p):
        sys.path.insert(0, _p)

import numpy as np
from contextlib import ExitStack

import concourse.bass as bass
import concourse.bacc as bacc
import concourse.tile as tile
import concourse.mybir as mybir
from concourse.bass_utils import run_bass_kernel_spmd

F32 = mybir.dt.float32
F32R = mybir.dt.float32r
AF = mybir.ActivationFunctionType
OP = mybir.AluOpType

P = 128
C = 1024          # model dim
KO = C // P       # 8 k-subtiles
B = 2
SEQ = 2048
TOKS = B * SEQ    # 4096
TB = 512          # token block (phase 1 / q blocks)
NTB = TOKS // TB  # 8
HD = 64
NQB = SEQ // TB   # 4 q-blocks per batch
NKT = SEQ // P    # 16 k-tiles per batch
NQT = SEQ // P    # 16 q row-tiles per batch (proj)
EPS = 1e-5
NCORES = 8


def _r(ap):
    return ap.bitcast(F32R)


def _emit(tc):
    nc = tc.nc
    xT = nc.dram_tensor("xT", [NTB, P, KO, TB], F32, kind="ExternalInput")
    w = nc.dram_tensor("w", [P, KO, 384], F32, kind="ExternalInput")
    bqkv = nc.dram_tensor("bqkv", [P, 3], F32, kind="ExternalInput")
    wp = nc.dram_tensor("wp", [P, C], F32, kind="ExternalInput")
    # aux: col 0 ones, col 1 eps; two [128, 66] stats selectors:
    # q-sel (cols 2:68): head A rows -> out row 0, B -> row 1, rest zero
    # k-sel (cols 68:134): head A rows -> out row 64, B -> row 65, rest zero
    aux = nc.dram_tensor("aux", [P, 134], F32, kind="ExternalInput")
    # rows2[h, s, :]: s=0 all-ones; s in 1..4: head-padded g/be rows
    # (row = [val|0] for head A, [0|val] for B) at partitions {0,1} and {64,65}
    rows2 = nc.dram_tensor("rows2", [66, 5, P], F32, kind="ExternalInput")
    bq2 = nc.dram_tensor("bq2", [P, 1], F32, kind="ExternalInput")
    bk2 = nc.dram_tensor("bk2", [P, 1], F32, kind="ExternalInput")
    idd = nc.dram_tensor("idd", [P, P], F32, kind="ExternalInput")      # identity
    y = nc.dram_tensor("y", [B, NQT, P, C], F32, kind="ExternalOutput")

    with ExitStack() as ctx:
        const = ctx.enter_context(tc.tile_pool(name="const", bufs=1))
        resid = ctx.enter_context(tc.tile_pool(name="resid", bufs=1))
        xst = ctx.enter_context(tc.tile_pool(name="xst", bufs=2))
        scratch = ctx.enter_context(tc.tile_pool(name="scratch", bufs=4))
        bcast = ctx.enter_context(tc.tile_pool(name="bcast", bufs=3))
        st1 = ctx.enter_context(tc.tile_pool(name="st1", bufs=4))
        stb = ctx.enter_context(tc.tile_pool(name="stb", bufs=3))
        ysb = ctx.enter_context(tc.tile_pool(name="ysb", bufs=3))
        psa = ctx.enter_context(tc.tile_pool(name="psa", bufs=2, space="PSUM"))
        pso = ctx.enter_context(tc.tile_pool(name="pso", bufs=2, space="PSUM"))
        psq = ctx.enter_context(tc.tile_pool(name="psq", bufs=1, space="PSUM"))

        # ---- constants ----
        w_sb = const.tile([P, KO, 384], F32)
        nc.sync.dma_start(_r(w_sb[:]), _r(w[:, :, :]))
        wp_sb = const.tile([P, C], F32)
        nc.sync.dma_start(_r(wp_sb[:]), _r(wp[:, :]))
        b_sb = const.tile([P, 3], F32)
        nc.sync.dma_start(b_sb[:], bqkv[:, :])
        ident = const.tile([P, P], F32)
        nc.sync.dma_start(ident[:], idd[:, :])
        aux_sb = const.tile([P, 134], F32)
        nc.sync.dma_start(_r(aux_sb[:]), _r(aux[:, :]))
        ones = aux_sb[:, 0:1]
        rows_sb = const.tile([66, 5, P], F32)
        nc.sync.dma_start(_r(rows_sb[:]), _r(rows2[:, :, :]))
        bq2_sb = const.tile([P, 1], F32)
        nc.sync.dma_start(bq2_sb[:], bq2[:, :])
        bk2_sb = const.tile([P, 1], F32)
        nc.sync.dma_start(bk2_sb[:], bk2[:, :])

        # ---- residents ----
        qT = resid.tile([P, TOKS], F32)   # heads 2c (rows 0:64) and 2c+1 (64:128)
        kT = resid.tile([P, TOKS], F32)
        vT = resid.tile([P, TOKS], F32)
        vtok = resid.tile([P, B * 2, NKT, HD + 1], F32)  # token-major V + ones col
        nc.vector.tensor_copy(_r(vtok[:, :, :, HD:HD + 1]),
                              ones.to_broadcast((P, B * 2, NKT, 1)))
        OT2 = resid.tile([P, B, SEQ], F32)  # normalized attention out, heads stacked

        # ---- phase 1: qkvT = w^T @ xT, biased; block-local LN stats+apply ----
        def emit_tb(tb):
            ts = slice(tb * TB, (tb + 1) * TB)
            xc = xst.tile([P, KO, TB], F32)
            nc.sync.dma_start(_r(xc[:, 0:KO // 2, :]), _r(xT[tb, :, 0:KO // 2, :]))
            nc.sync.dma_start(_r(xc[:, KO // 2:KO, :]), _r(xT[tb, :, KO // 2:KO, :]))
            for ct, dest in ((0, qT), (1, kT), (2, vT)):
                ps = psq.tile([P, TB], F32, tag="q")
                for ko in range(KO):
                    nc.tensor.matmul(
                        ps[:],
                        lhsT=_r(w_sb[:, ko, ct * P:(ct + 1) * P]),
                        rhs=_r(xc[:, ko, :]),
                        start=(ko == 0),
                        stop=(ko == KO - 1),
                    )
                dslc = dest[:, ts] if ct == 2 else _r(dest[:, ts])
                nc.scalar.activation(dslc, ps[:], AF.Identity,
                                     bias=b_sb[:, ct:ct + 1], scale=1.0)
            # Block-local LN of q and k. All four (tensor, head) instances are
            # partition-packed into one 2-bank stats psum tile via M=32
            # replicated ones matmuls (rows 0:32 qA | 32:64 qB | 64:96 kA |
            # 96:128 kB; free slots mu|msq), so the whole stats pipeline runs
            # as a handful of full-width DVE ops.
            sqq = scratch.tile([P, TB], F32, tag="sc")
            nc.scalar.activation(_r(sqq[:]), qT[:, ts], AF.Square)
            sqk = scratch.tile([P, TB], F32, tag="sc")
            nc.scalar.activation(_r(sqk[:]), kT[:, ts], AF.Square)
            sel_q = aux_sb[:, 2:68]
            sel_k = aux_sb[:, 68:134]
            stqk = psa.tile([66, 2, TB], F32, tag="a2")
            nc.tensor.matmul(stqk[:, 0, :], lhsT=_r(sel_q), rhs=_r(qT[:, ts]),
                             start=True, stop=False)
            nc.tensor.matmul(stqk[:, 0, :], lhsT=_r(sel_k), rhs=_r(kT[:, ts]),
                             start=False, stop=True)
            nc.tensor.matmul(stqk[:, 1, :], lhsT=_r(sel_q), rhs=_r(sqq[:]),
                             start=True, stop=False)
            nc.tensor.matmul(stqk[:, 1, :], lhsT=_r(sel_k), rhs=_r(sqk[:]),
                             start=False, stop=True)
            t_all = stb.tile([66, 2, TB], F32, tag="st")   # mu|msq -> nb|rs
            t_sq = stb.tile([66, TB], F32, tag="st2")
            nc.scalar.activation(_r(t_all[:, :, :]), stqk[:, :, :], AF.Identity,
                                 bias=0.0, scale=1.0 / HD)
            nc.vector.tensor_tensor(_r(t_sq[:]), t_all[:, 0, :], t_all[:, 0, :],
                                    OP.mult)
            nc.vector.tensor_tensor(_r(t_all[:, 1, :]), t_all[:, 1, :], t_sq[:],
                                    OP.subtract)
            nc.scalar.activation(_r(t_all[:, 1, :]), t_all[:, 1, :], AF.Sqrt,
                                 bias=aux_sb[0:66, 1:2])
            with nc.allow_low_precision(reason="fp32r feed to PE broadcast"):
                nc.vector.reciprocal(_r(t_all[:, 1, :]), t_all[:, 1, :])   # rs
            nc.vector.scalar_tensor_tensor(_r(t_all[:, 0, :]), t_all[:, 0, :],
                                           -1.0, t_all[:, 1, :],
                                           OP.mult, OP.mult)               # -mu*rs
            # Per-(partition,token) LN coefficients via K=1 outer-product
            # matmuls, with gamma/beta folded in:
            #   rbnb[:,0,:] = g (x) rs        rbnb[:,1,:] = g (x) nb + be (x) 1
            for src_t, gsl, bev, r0 in ((qT, 1, bq2_sb, 0),
                                        (kT, 3, bk2_sb, 64)):
                rbnb = psa.tile([P, 2, TB], F32, tag="a2",
                                name=f"rbnb_{tb}_{gsl}")
                nc.tensor.matmul(rbnb[:, 0, :],
                                 lhsT=_r(rows_sb[r0:r0 + 2, gsl, :]),
                                 rhs=_r(t_all[r0:r0 + 2, 1, :]),
                                 start=True, stop=True)
                nc.tensor.matmul(rbnb[:, 1, :],
                                 lhsT=_r(rows_sb[r0:r0 + 2, gsl, :]),
                                 rhs=_r(t_all[r0:r0 + 2, 0, :]),
                                 start=True, stop=True)
                tgt = src_t[:, ts]
                nc.vector.tensor_tensor(_r(tgt), tgt, rbnb[:, 0, :], OP.mult)
                nc.vector.scalar_tensor_tensor(_r(tgt), tgt, bev[:, :],
                                               rbnb[:, 1, :], OP.add, OP.add)
            # V transposes for this block's tokens (token-major V for O matmuls)
            vb2 = tb // (NTB // B)
            for h in range(2):
                hb = HD * h
                for kt in range((tb % 4) * 4, (tb % 4) * 4 + 4):
                    kts = slice(vb2 * SEQ + kt * P, vb2 * SEQ + (kt + 1) * P)
                    ps_t = pso.tile([P, HD], F32, tag="o")
                    nc.tensor.transpose(ps_t[:], vT[hb:hb + HD, kts],
                                        ident[hb:hb + HD, hb:hb + HD])
                    nc.vector.tensor_copy(_r(vtok[:, vb2 * 2 + h, kt, 0:HD]),
                                          ps_t[:])

        # ---- phase 2: attention ----
        def emit_attn(b2, qb):
            if True:
                qs = slice(b2 * SEQ + qb * TB, b2 * SEQ + (qb + 1) * TB)
                o_ps = [pso.tile([HD + 1, TB], F32, tag="o", name=f"o_{b2}_{qb}_{hh}")
                        for hh in range(2)]
                for kt in range(NKT):
                    kts = slice(b2 * SEQ + kt * P, b2 * SEQ + (kt + 1) * P)
                    s2 = psa.tile([P, 2, TB], F32, tag="a2")
                    for h in range(2):
                        hb = HD * h
                        nc.tensor.matmul(s2[:, h, :],
                                         lhsT=_r(kT[hb:hb + HD, kts]),
                                         rhs=_r(qT[hb:hb + HD, qs]),
                                         start=True, stop=True)
                    e2 = scratch.tile([P, 2, TB], F32, tag="sc2")
                    nc.scalar.activation(_r(e2[:]), s2[:], AF.Exp)
                    for h in range(2):
                        nc.tensor.matmul(o_ps[h][:],
                                         lhsT=_r(vtok[:, b2 * 2 + h, kt, :]),
                                         rhs=_r(e2[:, h, :]),
                                         start=(kt == 0), stop=(kt == NKT - 1))
                # normalize: reciprocal of denominators, broadcast, scale
                rc0 = st1.tile([1, TB], F32, tag="t1")
                rc1 = st1.tile([1, TB], F32, tag="t1")
                with nc.allow_low_precision(reason="fp32r feed to PE broadcast"):
                    nc.vector.reciprocal(_r(rc0[:]), o_ps[0][HD:HD + 1, :])
                    nc.vector.reciprocal(_r(rc1[:]), o_ps[1][HD:HD + 1, :])
                rbp0 = psa.tile([P, TB], F32, tag="rb", bufs=1)
                nc.tensor.matmul(rbp0[:], lhsT=_r(rows_sb[0:1, 0, :]),
                                 rhs=_r(rc0[:]), start=True, stop=True)
                rbp1 = psa.tile([P, TB], F32, tag="rb", bufs=1)
                nc.tensor.matmul(rbp1[:], lhsT=_r(rows_sb[0:1, 0, :]),
                                 rhs=_r(rc1[:]), start=True, stop=True)
                rb0 = bcast.tile([HD, TB], F32, tag="bc")
                nc.vector.tensor_copy(rb0[:], rbp0[0:HD, :])
                rb1 = bcast.tile([HD, TB], F32, tag="bc")
                nc.vector.tensor_copy(rb1[:], rbp1[0:HD, :])
                nc.vector.tensor_tensor(
                    _r(OT2[0:HD, b2, qb * TB:(qb + 1) * TB]),
                    o_ps[0][0:HD, :], rb0[:], OP.mult)
                ob = ysb.tile([HD, TB], F32, tag="ob")
                nc.vector.tensor_tensor(_r(ob[:]), o_ps[1][0:HD, :],
                                        rb1[:], OP.mult)
                nc.sync.dma_start(
                    _r(OT2[HD:P, b2, qb * TB:(qb + 1) * TB]), _r(ob[:]))

        # ---- phase 3: projection partial ----
        def emit_proj(b2, qt):
            if True:
                yt = ysb.tile([P, C], F32, tag="yt", name=f"yt_{b2}_{qt}")
                for half in range(2):
                    pp = psq.tile([P, TB], F32, tag="q")
                    nc.tensor.matmul(
                        pp[:],
                        lhsT=_r(OT2[:, b2, qt * P:(qt + 1) * P]),
                        rhs=_r(wp_sb[:, half * TB:(half + 1) * TB]),
                        start=True, stop=True)
                    nc.vector.tensor_copy(yt[:, half * TB:(half + 1) * TB], pp[:])
                nc.sync.dma_start(y[b2, qt, :, :], yt[:])

        # ---- interleaved emission: attention(b2=0) woven into phase-1 tail,
        # proj(b2=0) woven into attention(b2=1), so the scheduler can fill
        # engine idle across phase boundaries ----
        for tb in range(NTB):
            emit_tb(tb)
        for b2 in range(B):
            for qb in range(NQB):
                emit_attn(b2, qb)
        for b2 in range(B):
            for qt in range(NQT):
                emit_proj(b2, qt)


_NC_CACHE = None


def build_nc():
    global _NC_CACHE
    if _NC_CACHE is None:
        nc = bacc.Bacc("TRN2", target_bir_lowering=False, debug=False)
        with tile.TileContext(nc) as tc:
            _emit(tc)
        nc.compile()
        _NC_CACHE = nc
    return _NC_CACHE


def make_in_maps(x, w_qkv, b_qkv, g_q, be_q, g_k, be_k, w_proj):
    x2 = np.ascontiguousarray(np.asarray(x, np.float32).reshape(TOKS, C))
    # xT[tb, p, ko, t] = x2[tb*TB + t, ko*128 + p]
    xT_h = np.ascontiguousarray(
        x2.T.reshape(KO, P, NTB, TB).transpose(2, 1, 0, 3))
    w_qkv = np.asarray(w_qkv, np.float32)
    b_qkv = np.asarray(b_qkv, np.float32)
    g_q = np.asarray(g_q, np.float32)
    be_q = np.asarray(be_q, np.float32)
    g_k = np.asarray(g_k, np.float32)
    be_k = np.asarray(be_k, np.float32)
    w_proj = np.asarray(w_proj, np.float32)

    aux_h = np.zeros((P, 134), np.float32)
    aux_h[:, 0] = 1.0
    aux_h[:, 1] = EPS
    aux_h[0:HD, 2] = 1.0         # q-sel: head A -> row 0
    aux_h[HD:P, 3] = 1.0         # q-sel: head B -> row 1
    aux_h[0:HD, 68 + 64] = 1.0   # k-sel: head A -> row 64
    aux_h[HD:P, 68 + 65] = 1.0   # k-sel: head B -> row 65
    rows_h = np.zeros((66, 5, P), np.float32)
    rows_h[:, 0, :] = 1.0
    for s, vec in ((1, g_q / 8.0), (2, be_q / 8.0), (3, g_k), (4, be_k)):
        for r in (0, 64):
            rows_h[r, s, 0:HD] = vec
            rows_h[r + 1, s, HD:P] = vec
    bq2_h = np.ascontiguousarray(np.tile(be_q / 8.0, 2).reshape(P, 1))
    bk2_h = np.ascontiguousarray(np.tile(be_k, 2).reshape(P, 1))
    idd_h = np.ascontiguousarray(np.eye(P, dtype=np.float32))
    in_maps = []
    for c in range(NCORES):
        cs = slice(P * c, P * (c + 1))
        wcat = np.concatenate(
            [w_qkv[:, 0:C][:, cs], w_qkv[:, C:2 * C][:, cs], w_qkv[:, 2 * C:3 * C][:, cs]],
            axis=1)  # [1024, 384]
        w_h = np.ascontiguousarray(wcat.reshape(KO, P, 384).transpose(1, 0, 2))
        bcat = np.concatenate(
            [b_qkv[0:C][cs], b_qkv[C:2 * C][cs], b_qkv[2 * C:3 * C][cs]])
        b_h = np.ascontiguousarray(bcat.reshape(3, P).T)
        wp_h = np.ascontiguousarray(w_proj[cs, :])
        in_maps.append({
            "xT": xT_h, "w": w_h, "bqkv": b_h,
            "wp": wp_h, "aux": aux_h, "rows2": rows_h, "idd": idd_h,
            "bq2": bq2_h, "bk2": bk2_h,
        })
    return in_maps


def kernel(x, w_qkv, b_qkv, g_q, be_q, g_k, be_k, w_proj, b_proj, **run_kwargs):
    in_maps = make_in_maps(x, w_qkv, b_qkv, g_q, be_q, g_k, be_k, w_proj)
    nc = build_nc()
    res = run_bass_kernel_spmd(nc, in_maps, list(range(NCORES)), **run_kwargs)
    acc = np.zeros((TOKS, C), np.float64)
    for r in res.results:
        acc += r["y"].reshape(TOKS, C)
    out = acc + np.asarray(b_proj, np.float32)
    out = out.astype(np.float32).reshape(B, SEQ, C)
    kernel.last_result = res
    return out
